# revision 1
# baseline (speedup 1.0000x reference)
"""Embedding lookup + small linear projection on 8 Trainium2 NeuronCores.

Computation (full problem):
    rows = user_repost_matrix[input.reshape(-1)]      # [12800, 2000] f32
    out  = rows @ W.T + b                             # [12800, 8]
    out.reshape(64, 200, 8)

Distribution strategy: pure data-parallel over the 12800 tokens (1600 per
core). The embedding table is replicated into every core's DRAM, so no
collectives are needed: per-core HBM gather traffic (1600 rows x 8KB =
12.8MB) is identical to a row-sharded layout with all-to-all, minus the
communication.

Per-core device kernel (Tile framework), per 128-token tile (13 tiles):
  1. gpsimd.indirect_dma_start gathers 128 table rows -> SBUF R [128, 2000]
  2. PE transposes 16 chunks of [128, 125] f32 -> PSUM [125, 128] (exact)
  3. Split each transposed chunk into bf16 hi + bf16 residual during the
     PSUM->SBUF copies:  RTh = bf16(psum); RTl = bf16(psum - RTh)
  4. Project with two bf16 matmuls per chunk (3-term compensated product,
     ~1e-5 relative error, ~4x cheaper on PE than fp32):
        c[128,16] += RTh^T @ [W2h | W2l]      (hh and h*lo terms)
        c[128,:8] += RTl^T @ W2h              (lo*h term)
  5. C = c[:, :8] + c[:, 8:] + bias on DVE, DMA result slice to DRAM
"""

import sys

if "/opt/trn_rl_repo" not in sys.path:
    sys.path.insert(0, "/opt/trn_rl_repo")

import ml_dtypes
import numpy as np

import concourse.bass as bass
import concourse.tile as tile
from concourse import bacc, mybir
from concourse.bass_utils import run_bass_kernel_spmd
from concourse.masks import make_identity

NTOKEN = 100000
D = 2000
J = 8
B, L = 64, 200
N_CORES = 8
TOK = B * L                      # 12800
PER_CORE = TOK // N_CORES        # 1600
P = 128
TILES = (PER_CORE + P - 1) // P  # 13 (last tile is half-padded)
PAD = TILES * P                  # 1664
KCH = 16                         # feature chunks
KC = D // KCH                    # 125

F32 = mybir.dt.float32
BF16 = mybir.dt.bfloat16
I32 = mybir.dt.int32

_cached = None


def _build():
    """Build + compile the SPMD Bass module once."""
    nc = bacc.Bacc(
        "TRN2", target_bir_lowering=False, debug=False, num_devices=N_CORES
    )
    table = nc.dram_tensor("table", [NTOKEN, D], F32, kind="ExternalInput").ap()
    idx = nc.dram_tensor("idx", [P, TILES], I32, kind="ExternalInput").ap()
    # w2hl[p, k*16 + j]     = bf16(W.T)[k*125 + p, j]          (hi part)
    # w2hl[p, k*16 + 8 + j] = bf16(W.T - hi)[k*125 + p, j]     (lo part)
    w2hl = nc.dram_tensor("w2hl", [KC, KCH * 2 * J], BF16, kind="ExternalInput").ap()
    bias = nc.dram_tensor("bias", [P, J], F32, kind="ExternalInput").ap()
    out = nc.dram_tensor("out", [PAD, J], F32, kind="ExternalOutput").ap()

    with tile.TileContext(nc) as tc:
        with (
            tc.tile_pool(name="const", bufs=1) as cpool,
            tc.tile_pool(name="rows", bufs=4) as rpool,
            tc.tile_pool(name="tpsum", bufs=6, space="PSUM") as tppool,
            tc.tile_pool(name="rth", bufs=6) as rthpool,
            tc.tile_pool(name="rtl", bufs=6) as rtlpool,
            tc.tile_pool(name="cpsum", bufs=2, space="PSUM") as cppool,
            tc.tile_pool(name="o", bufs=2) as opool,
        ):
            idx_sb = cpool.tile([P, TILES], I32)
            nc.sync.dma_start(idx_sb[:], idx[:])
            w2_sb = cpool.tile([KC, KCH * 2 * J], BF16)
            nc.sync.dma_start(w2_sb[:], w2hl[:])
            bias_sb = cpool.tile([P, J], F32)
            nc.sync.dma_start(bias_sb[:], bias[:])
            ident = cpool.tile([P, P], F32)
            make_identity(nc, ident[:])

            for i in range(TILES):
                r = rpool.tile([P, D], F32)
                nc.gpsimd.indirect_dma_start(
                    out=r[:],
                    out_offset=None,
                    in_=table[:],
                    in_offset=bass.IndirectOffsetOnAxis(
                        ap=idx_sb[:, i : i + 1], axis=0
                    ),
                )
                c_ps = cppool.tile([P, 2 * J], F32, space="PSUM")
                for k in range(KCH):
                    t_ps = tppool.tile([KC, P], F32, space="PSUM")
                    nc.tensor.transpose(
                        out=t_ps[:],
                        in_=r[:, k * KC : (k + 1) * KC],
                        identity=ident[:],
                    )
                    rth = rthpool.tile([KC, P], BF16)
                    # round-to-bf16 copy; alternate DVE/ACT (ACT copy is
                    # ~2x DVE, so give ACT only every other one)
                    if k % 2 == 0:
                        nc.scalar.copy(rth[:], t_ps[:])
                    else:
                        nc.vector.tensor_copy(rth[:], t_ps[:])
                    rtl = rtlpool.tile([KC, P], BF16)
                    nc.vector.tensor_tensor(
                        out=rtl[:],
                        in0=t_ps[:],
                        in1=rth[:],
                        op=mybir.AluOpType.subtract,
                    )
                    nc.tensor.matmul(
                        out=c_ps[:],
                        lhsT=rth[:],
                        rhs=w2_sb[:, k * 2 * J : (k + 1) * 2 * J],
                        start=(k == 0),
                        stop=False,
                        skip_group_check=True,
                    )
                    nc.tensor.matmul(
                        out=c_ps[:, :J],
                        lhsT=rtl[:],
                        rhs=w2_sb[:, k * 2 * J : k * 2 * J + J],
                        start=False,
                        stop=(k == KCH - 1),
                        skip_group_check=True,
                    )
                # combine hh + (hl + lh-term) + bias; one PSUM operand per op
                o = opool.tile([P, J], F32)
                nc.vector.tensor_add(o[:], c_ps[:, :J], bias_sb[:])
                nc.vector.tensor_add(o[:], o[:], c_ps[:, J:])
                nc.sync.dma_start(out[i * P : (i + 1) * P, :], o[:])

    nc.compile()
    return nc


def _get_nc():
    global _cached
    if _cached is None:
        _cached = _build()
    return _cached


def _prep_in_maps(input, user_repost_matrix, W, b):
    idx_full = np.asarray(input).reshape(-1).astype(np.int32)
    table = np.ascontiguousarray(np.asarray(user_repost_matrix, dtype=np.float32))
    Wt = np.asarray(W, dtype=np.float32).T                      # [2000, 8]
    # chunked layout: wc[k][p, j] = W.T[k*125+p, j]
    wc = Wt.reshape(KCH, KC, J)                                  # [16, 125, 8]
    wh = wc.astype(ml_dtypes.bfloat16)
    wl = (wc - wh.astype(np.float32)).astype(ml_dtypes.bfloat16)
    # w2hl[p, k, 0:8] = wh[k, p, :]; w2hl[p, k, 8:16] = wl[k, p, :]
    w2hl = np.concatenate([wh, wl], axis=2)                      # [16, 125, 16]
    w2hl = np.ascontiguousarray(
        w2hl.transpose(1, 0, 2).reshape(KC, KCH * 2 * J)
    )
    bias = np.ascontiguousarray(
        np.broadcast_to(np.asarray(b, dtype=np.float32).reshape(1, J), (P, J))
    )
    in_maps = []
    for c in range(N_CORES):
        chunk = idx_full[c * PER_CORE : (c + 1) * PER_CORE]
        padded = np.zeros(PAD, np.int32)
        padded[:PER_CORE] = chunk
        # idx_dram[p, i] = core-local token i*128 + p
        idx_arr = np.ascontiguousarray(padded.reshape(TILES, P).T)
        in_maps.append(
            {"table": table, "idx": idx_arr, "w2hl": w2hl, "bias": bias}
        )
    return in_maps


def _run(in_maps, trace=False, **kw):
    nc = _get_nc()
    return run_bass_kernel_spmd(
        nc, in_maps, list(range(N_CORES)), trace=trace, **kw
    )


def _unshard(results):
    parts = [results[c]["out"][:PER_CORE] for c in range(N_CORES)]
    return np.concatenate(parts, axis=0).reshape(B, L, J).astype(np.float32)


def kernel(input, user_repost_matrix, W, b):
    in_maps = _prep_in_maps(input, user_repost_matrix, W, b)
    res = _run(in_maps)
    return _unshard(res.results)



# revision 6
# speedup vs baseline: 1.8388x; 1.8388x over previous
"""Embedding lookup + small linear projection on 8 Trainium2 NeuronCores.

Computation (full problem):
    rows = user_repost_matrix[input.reshape(-1)]      # [12800, 2000] f32
    out  = rows @ W.T + b                             # [12800, 8]
    out.reshape(64, 200, 8)

Distribution: the table is sharded row-wise. The host sorts the 12800
tokens by index and hands core c the c-th run of 1664 sorted tokens
(core 7 gets the remaining 1152 plus padding), so each core's indices
fall in one contiguous table window. Each core is staged a fixed-shape
[16384, 2048] bf16 slice of the table covering its window, and local
indices fit int16.

Per-core device kernel (Tile framework):
  1. gpsimd.dma_gather(transpose=True) pulls its rows from DRAM directly
     into chunk-transposed SBUF layout G[p, c, t] = row_t[c*128 + p]
     (bf16, 16 chunks of 128). No on-chip transpose work at all.
  2. Per 128-512-token group: 16 accumulating PE matmuls
     psum[8, T] += W_chunk[128, 8].T @ G[:, c, group]   (bf16, f32 acc)
  3. DVE copies psum -> SBUF, DMA to DRAM out [8, 1664] (transposed).

Host post-pass: inverse-permute token order, transpose, add bias. Any
token whose index fell outside its core's staged window (impossible for
uniform data, possible for adversarial distributions) is recomputed on
the host in f32 as a correctness fallback.

Precision: table and W are bf16 (round-to-nearest), accumulation in
f32 PSUM -> rel err ~2e-3, well inside the 2e-2 gate.
"""

import sys

if "/opt/trn_rl_repo" not in sys.path:
    sys.path.insert(0, "/opt/trn_rl_repo")

import ml_dtypes
import numpy as np

import concourse.tile as tile
from concourse import bacc, mybir
from concourse.bass_utils import run_bass_kernel_spmd

NTOKEN = 100000
D = 2000
D2 = 2048                        # feature dim padded to 16*128
J = 8
B, L = 64, 200
N_CORES = 8
TOK = B * L                      # 12800
NI = 1664                        # tokens per core (13*128)
S = 16384                        # staged table rows per core
KCH = 16                         # feature chunks of 128
GROUPS = (256, 256, 512, 512, 128)   # gather/matmul group sizes, sum == NI

F32 = mybir.dt.float32
BF16 = mybir.dt.bfloat16
I16 = mybir.dt.int16

_cached = None


def _build():
    """Build + compile the SPMD Bass module once."""
    nc = bacc.Bacc(
        "TRN2", target_bir_lowering=False, debug=False, num_devices=N_CORES
    )
    table = nc.dram_tensor("table", [S, D2], BF16, kind="ExternalInput").ap()
    idx = nc.dram_tensor("idx", [128, NI // 16], I16, kind="ExternalInput").ap()
    # w[p, c*8 + j] = bf16(W.T padded)[c*128 + p, j]
    w = nc.dram_tensor("w", [128, KCH * J], BF16, kind="ExternalInput").ap()
    out = nc.dram_tensor("out", [J, NI], F32, kind="ExternalOutput").ap()

    with tile.TileContext(nc) as tc:
        with (
            tc.tile_pool(name="const", bufs=1) as cpool,
            tc.tile_pool(name="g", bufs=1) as gpool,
            tc.tile_pool(name="ps", bufs=2, space="PSUM") as pspool,
            tc.tile_pool(name="o", bufs=2) as opool,
        ):
            idx_sb = cpool.tile([128, NI // 16], I16)
            nc.sync.dma_start(idx_sb[:], idx[:])
            w_sb = cpool.tile([128, KCH * J], BF16)
            nc.sync.dma_start(w_sb[:], w[:])

            gtiles = []
            off = 0
            for gi, n in enumerate(GROUPS):
                g = gpool.tile([128, KCH, n], BF16, name=f"G{gi}")
                nc.gpsimd.dma_gather(
                    g[:],
                    table[:],
                    idx_sb[:, off // 16 : (off + n) // 16],
                    n,
                    n,
                    D2,
                    transpose=True,
                )
                gtiles.append(g)
                off += n

            off = 0
            for gi, n in enumerate(GROUPS):
                g = gtiles[gi]
                c_ps = pspool.tile([J, 512], F32, space="PSUM", name="c_ps")
                for c in range(KCH):
                    nc.tensor.matmul(
                        out=c_ps[:, :n],
                        lhsT=w_sb[:, c * J : (c + 1) * J],
                        rhs=g[:, c, :],
                        start=(c == 0),
                        stop=(c == KCH - 1),
                    )
                o = opool.tile([J, 512], F32, name="o")[:, :n]
                nc.vector.tensor_copy(o[:], c_ps[:, :n])
                nc.sync.dma_start(out[:, off : off + n], o[:])
                off += n

    nc.compile()
    return nc


def _get_nc():
    global _cached
    if _cached is None:
        _cached = _build()
    return _cached


def _prep_in_maps(input, user_repost_matrix, W, b):
    idx_full = np.asarray(input).reshape(-1).astype(np.int64)
    table_f32 = np.asarray(user_repost_matrix, dtype=np.float32)
    W_f32 = np.asarray(W, dtype=np.float32)
    b_f32 = np.asarray(b, dtype=np.float32)

    # bf16 table, feature dim padded to 2048
    tbl = np.zeros((NTOKEN, D2), dtype=ml_dtypes.bfloat16)
    tbl[:, :D] = table_f32.astype(ml_dtypes.bfloat16)

    # W tile: w[p, c*8+j] = Wt_pad[c*128+p, j]
    wt = np.zeros((D2, J), dtype=np.float32)
    wt[:D] = W_f32.T
    w_tile = np.ascontiguousarray(
        wt.astype(ml_dtypes.bfloat16)
        .reshape(KCH, 128, J)
        .transpose(1, 0, 2)
        .reshape(128, KCH * J)
    )

    order = np.argsort(idx_full, kind="stable")
    idx_sorted = idx_full[order]

    in_maps = []
    bases = []
    oob = []                      # (core, slot) of out-of-window tokens
    for c in range(N_CORES):
        lo = c * NI
        hi = min(lo + NI, TOK)
        cnt = hi - lo
        gidx = np.empty(NI, np.int64)
        gidx[:cnt] = idx_sorted[lo:hi]
        gidx[cnt:] = gidx[cnt - 1]
        base = int(min(gidx[0], NTOKEN - S))
        loc = gidx - base
        bad = (loc < 0) | (loc >= S)
        if bad.any():
            for slot in np.nonzero(bad)[0]:
                oob.append((c, int(slot)))
            loc = np.clip(loc, 0, S - 1)
        loc16 = loc.astype(np.int16)
        # idx tile: slot i -> [g*16 + i%16, i//16], replicated over 8 groups
        idx_tile = np.tile(
            np.ascontiguousarray(loc16.reshape(NI // 16, 16).T), (8, 1)
        )
        in_maps.append(
            {
                "table": np.ascontiguousarray(tbl[base : base + S]),
                "idx": idx_tile,
                "w": w_tile,
            }
        )
        bases.append(base)

    ctx = {
        "order": order,
        "oob": oob,
        "idx_full": idx_full,
        "table_f32": table_f32,
        "W_f32": W_f32,
        "b_f32": b_f32,
    }
    return in_maps, ctx


def _run(in_maps, trace=False, **kw):
    nc = _get_nc()
    return run_bass_kernel_spmd(
        nc, in_maps, list(range(N_CORES)), trace=trace, **kw
    )


def _unshard(results, ctx):
    order = ctx["order"]
    sorted_out = np.concatenate(
        [results[c]["out"] for c in range(N_CORES)], axis=1
    )[:, :TOK].T.astype(np.float32)          # [12800, 8] in sorted order
    final = np.empty((TOK, J), np.float32)
    final[order] = sorted_out
    # host f32 fallback for tokens outside their core's staged window
    for c, slot in ctx["oob"]:
        k = c * NI + slot
        if k < TOK:
            tok = order[k]
            final[tok] = ctx["table_f32"][ctx["idx_full"][tok]] @ ctx["W_f32"].T
    final += ctx["b_f32"].reshape(1, J)
    return final.reshape(B, L, J)


def kernel(input, user_repost_matrix, W, b):
    in_maps, ctx = _prep_in_maps(input, user_repost_matrix, W, b)
    res = _run(in_maps)
    return _unshard(res.results, ctx)


# revision 11
# speedup vs baseline: 2.1279x; 1.1572x over previous
"""Embedding lookup + small linear projection on 8 Trainium2 NeuronCores.

Computation (full problem):
    rows = user_repost_matrix[input.reshape(-1)]      # [12800, 2000] f32
    out  = rows @ W.T + b                             # [12800, 8]
    out.reshape(64, 200, 8)

Distribution: the table is sharded row-wise. The host sorts the 12800
tokens by index and hands core c the c-th run of 1664 sorted tokens
(core 7 gets the remaining 1152 plus padding), so each core's indices
fall in one contiguous table window. Each core is staged a fixed-shape
[16384, 2048] bf16 slice of the table covering its window, and local
indices fit int16.

Per-core device kernel (Tile framework):
  1. gpsimd.dma_gather(transpose=True) pulls its rows from DRAM directly
     into chunk-transposed SBUF layout G[p, c, t] = row_t[c*128 + p]
     (bf16, 16 chunks of 128). No on-chip transpose work at all.
  2. Per 128-512-token group: 16 accumulating PE matmuls
     psum[8, T] += W_chunk[128, 8].T @ G[:, c, group]   (bf16, f32 acc)
  3. DVE copies psum -> SBUF, DMA to DRAM out [8, 1664] (transposed).

Host post-pass: inverse-permute token order, transpose, add bias. Any
token whose index fell outside its core's staged window (impossible for
uniform data, possible for adversarial distributions) is recomputed on
the host in f32 as a correctness fallback.

Precision: table and W are bf16 (round-to-nearest), accumulation in
f32 PSUM -> rel err ~2e-3, well inside the 2e-2 gate.
"""

import sys

if "/opt/trn_rl_repo" not in sys.path:
    sys.path.insert(0, "/opt/trn_rl_repo")

import ml_dtypes
import numpy as np

import concourse.tile as tile
from concourse import bacc, library_config, mybir
from concourse.bass_utils import run_bass_kernel_spmd

NTOKEN = 100000
D = 2000
D2 = 2048                        # feature dim padded to 16*128
J = 8
B, L = 64, 200
N_CORES = 8
TOK = B * L                      # 12800
NI = 1664                        # tokens per core (13*128)
S = 16384                        # staged table rows per core
KCH = 16                         # feature chunks of 128
# gather/matmul group sizes, sum == NI; small first group for a fast
# pipeline start, small last group for a short tail
GROUPS = (128, 512, 512, 384, 128)

F32 = mybir.dt.float32
BF16 = mybir.dt.bfloat16
I16 = mybir.dt.int16

_cached = None


def _build():
    """Build + compile the SPMD Bass module once."""
    nc = bacc.Bacc(
        "TRN2",
        target_bir_lowering=False,
        debug=False,
        num_devices=N_CORES,
        num_swdge_queues=4,
    )
    table = nc.dram_tensor("table", [S, D2], BF16, kind="ExternalInput").ap()
    idx = nc.dram_tensor("idx", [128, NI // 16], I16, kind="ExternalInput").ap()
    # w[p, c*8 + j] = bf16(W.T padded)[c*128 + p, j]
    w = nc.dram_tensor("w", [128, KCH * J], BF16, kind="ExternalInput").ap()
    out = nc.dram_tensor("out", [J, NI], F32, kind="ExternalOutput").ap()

    with tile.TileContext(nc) as tc:
        with (
            tc.tile_pool(name="const", bufs=1) as cpool,
            tc.tile_pool(name="g", bufs=1) as gpool,
            tc.tile_pool(name="ps", bufs=2, space="PSUM") as pspool,
            tc.tile_pool(name="o", bufs=2) as opool,
        ):
            # kick off the Q7 gather-ucode IRAM load right away so it
            # overlaps the framework preamble instead of gating gather 0
            nc.gpsimd.load_library(library_config.mlp)

            idx_sb = cpool.tile([128, NI // 16], I16)
            nc.sync.dma_start(idx_sb[:], idx[:])
            w_sb = cpool.tile([128, KCH * J], BF16)
            nc.sync.dma_start(w_sb[:], w[:])

            gtiles = []
            off = 0
            for gi, n in enumerate(GROUPS):
                g = gpool.tile([128, KCH, n], BF16, name=f"G{gi}")
                nc.gpsimd.dma_gather(
                    g[:],
                    table[:],
                    idx_sb[:, off // 16 : (off + n) // 16],
                    n,
                    n,
                    D2,
                    transpose=True,
                    queue_num=gi % 4,
                )
                gtiles.append(g)
                off += n

            off = 0
            for gi, n in enumerate(GROUPS):
                g = gtiles[gi]
                c_ps = pspool.tile([J, 512], F32, space="PSUM", name="c_ps")
                for c in range(KCH):
                    nc.tensor.matmul(
                        out=c_ps[:, :n],
                        lhsT=w_sb[:, c * J : (c + 1) * J],
                        rhs=g[:, c, :],
                        start=(c == 0),
                        stop=(c == KCH - 1),
                    )
                o = opool.tile([J, 512], F32, name="o")[:, :n]
                nc.vector.tensor_copy(o[:], c_ps[:, :n])
                nc.sync.dma_start(out[:, off : off + n], o[:])
                off += n

    nc.compile()
    return nc


def _get_nc():
    global _cached
    if _cached is None:
        _cached = _build()
    return _cached


def _prep_in_maps(input, user_repost_matrix, W, b):
    idx_full = np.asarray(input).reshape(-1).astype(np.int64)
    table_f32 = np.asarray(user_repost_matrix, dtype=np.float32)
    W_f32 = np.asarray(W, dtype=np.float32)
    b_f32 = np.asarray(b, dtype=np.float32)

    # bf16 table, feature dim padded to 2048
    tbl = np.zeros((NTOKEN, D2), dtype=ml_dtypes.bfloat16)
    tbl[:, :D] = table_f32.astype(ml_dtypes.bfloat16)

    # W tile: w[p, c*8+j] = Wt_pad[c*128+p, j]
    wt = np.zeros((D2, J), dtype=np.float32)
    wt[:D] = W_f32.T
    w_tile = np.ascontiguousarray(
        wt.astype(ml_dtypes.bfloat16)
        .reshape(KCH, 128, J)
        .transpose(1, 0, 2)
        .reshape(128, KCH * J)
    )

    order = np.argsort(idx_full, kind="stable")
    idx_sorted = idx_full[order]

    in_maps = []
    bases = []
    oob = []                      # (core, slot) of out-of-window tokens
    for c in range(N_CORES):
        lo = c * NI
        hi = min(lo + NI, TOK)
        cnt = hi - lo
        gidx = np.empty(NI, np.int64)
        gidx[:cnt] = idx_sorted[lo:hi]
        gidx[cnt:] = gidx[cnt - 1]
        base = int(min(gidx[0], NTOKEN - S))
        loc = gidx - base
        bad = (loc < 0) | (loc >= S)
        if bad.any():
            for slot in np.nonzero(bad)[0]:
                oob.append((c, int(slot)))
            loc = np.clip(loc, 0, S - 1)
        loc16 = loc.astype(np.int16)
        # idx tile: slot i -> [g*16 + i%16, i//16], replicated over 8 groups
        idx_tile = np.tile(
            np.ascontiguousarray(loc16.reshape(NI // 16, 16).T), (8, 1)
        )
        in_maps.append(
            {
                "table": np.ascontiguousarray(tbl[base : base + S]),
                "idx": idx_tile,
                "w": w_tile,
            }
        )
        bases.append(base)

    ctx = {
        "order": order,
        "oob": oob,
        "idx_full": idx_full,
        "table_f32": table_f32,
        "W_f32": W_f32,
        "b_f32": b_f32,
    }
    return in_maps, ctx


def _run(in_maps, trace=False, **kw):
    nc = _get_nc()
    return run_bass_kernel_spmd(
        nc, in_maps, list(range(N_CORES)), trace=trace, **kw
    )


def _unshard(results, ctx):
    order = ctx["order"]
    sorted_out = np.concatenate(
        [results[c]["out"] for c in range(N_CORES)], axis=1
    )[:, :TOK].T.astype(np.float32)          # [12800, 8] in sorted order
    final = np.empty((TOK, J), np.float32)
    final[order] = sorted_out
    # host f32 fallback for tokens outside their core's staged window
    for c, slot in ctx["oob"]:
        k = c * NI + slot
        if k < TOK:
            tok = order[k]
            final[tok] = ctx["table_f32"][ctx["idx_full"][tok]] @ ctx["W_f32"].T
    final += ctx["b_f32"].reshape(1, J)
    return final.reshape(B, L, J)


def kernel(input, user_repost_matrix, W, b):
    in_maps, ctx = _prep_in_maps(input, user_repost_matrix, W, b)
    res = _run(in_maps)
    return _unshard(res.results, ctx)


# revision 16
# speedup vs baseline: 2.3026x; 1.0821x over previous
"""Embedding lookup + small linear projection on 8 Trainium2 NeuronCores.

Computation (full problem):
    rows = user_repost_matrix[input.reshape(-1)]      # [12800, 2000] f32
    out  = rows @ W.T + b                             # [12800, 8]
    out.reshape(64, 200, 8)

Distribution: the table is sharded row-wise. The host sorts the 12800
tokens by index and hands core c the c-th run of 1664 sorted tokens
(core 7 gets the remaining 1152 plus padding), so each core's indices
fall in one contiguous table window. Each core is staged a fixed-shape
[16384, 2048] bf16 slice of the table covering its window, and local
indices fit int16.

Per-core device kernel (Tile framework):
  1. gpsimd.dma_gather(transpose=True) pulls its rows from DRAM directly
     into chunk-transposed SBUF layout G[p, c, t] = row_t[c*128 + p]
     (bf16, 16 chunks of 128). No on-chip transpose work at all.
  2. Per 128-512-token group: 16 accumulating PE matmuls
     psum[8, T] += W_chunk[128, 8].T @ G[:, c, group]   (bf16, f32 acc)
  3. DVE copies psum -> SBUF, DMA to DRAM out [8, 1664] (transposed).

Host post-pass: inverse-permute token order, transpose, add bias. Any
token whose index fell outside its core's staged window (impossible for
uniform data, possible for adversarial distributions) is recomputed on
the host in f32 as a correctness fallback.

Precision: table and W are bf16 (round-to-nearest), accumulation in
f32 PSUM -> rel err ~2e-3, well inside the 2e-2 gate.
"""

import sys

if "/opt/trn_rl_repo" not in sys.path:
    sys.path.insert(0, "/opt/trn_rl_repo")

import ml_dtypes
import numpy as np

import concourse.tile as tile
from concourse import bacc, library_config, mybir
from concourse.bass_utils import run_bass_kernel_spmd

NTOKEN = 100000
D = 2000
D2 = 2048                        # feature dim padded to 16*128
J = 8
B, L = 64, 200
N_CORES = 8
TOK = B * L                      # 12800
NI = 1664                        # tokens per core (13*128)
S = 16384                        # staged table rows per core
KCH = 16                         # feature chunks of 128
# gather/matmul group sizes, sum == NI. One SWDGE queue drains FIFO, so
# completions are progressive; 256-row groups keep descriptor-gen ahead
# of the drain without overflowing the ring. Small last group shortens
# the matmul tail after the final transfer.
GROUPS = (256, 256, 256, 256, 256, 256, 128)

F32 = mybir.dt.float32
BF16 = mybir.dt.bfloat16
I16 = mybir.dt.int16

_cached = None


def _build():
    """Build + compile the SPMD Bass module once."""
    nc = bacc.Bacc(
        "TRN2",
        target_bir_lowering=False,
        debug=False,
        num_devices=N_CORES,
    )
    table = nc.dram_tensor("table", [S, D2], BF16, kind="ExternalInput").ap()
    idx = nc.dram_tensor("idx", [128, NI // 16], I16, kind="ExternalInput").ap()
    # w[p, c*8 + j] = bf16(W.T padded)[c*128 + p, j]
    w = nc.dram_tensor("w", [128, KCH * J], BF16, kind="ExternalInput").ap()
    out = nc.dram_tensor("out", [J, NI], F32, kind="ExternalOutput").ap()

    with tile.TileContext(nc) as tc:
        with (
            tc.tile_pool(name="const", bufs=1) as cpool,
            tc.tile_pool(name="g", bufs=1) as gpool,
            tc.tile_pool(name="ps", bufs=2, space="PSUM") as pspool,
            tc.tile_pool(name="o", bufs=2) as opool,
        ):
            # kick off the Q7 gather-ucode IRAM load right away so it
            # overlaps the framework preamble instead of gating gather 0
            nc.gpsimd.load_library(library_config.mlp)

            idx_sb = cpool.tile([128, NI // 16], I16)
            nc.sync.dma_start(idx_sb[:], idx[:])
            w_sb = cpool.tile([128, KCH * J], BF16)
            nc.sync.dma_start(w_sb[:], w[:])

            gtiles = []
            off = 0
            for gi, n in enumerate(GROUPS):
                g = gpool.tile([128, KCH, n], BF16, name=f"G{gi}")
                nc.gpsimd.dma_gather(
                    g[:],
                    table[:],
                    idx_sb[:, off // 16 : (off + n) // 16],
                    n,
                    n,
                    D2,
                    transpose=True,
                )
                gtiles.append(g)
                off += n

            off = 0
            for gi, n in enumerate(GROUPS):
                g = gtiles[gi]
                c_ps = pspool.tile([J, 256], F32, space="PSUM", name="c_ps")
                for c in range(KCH):
                    nc.tensor.matmul(
                        out=c_ps[:, :n],
                        lhsT=w_sb[:, c * J : (c + 1) * J],
                        rhs=g[:, c, :],
                        start=(c == 0),
                        stop=(c == KCH - 1),
                    )
                o = opool.tile([J, 256], F32, name="o")[:, :n]
                nc.vector.tensor_copy(o[:], c_ps[:, :n])
                nc.sync.dma_start(out[:, off : off + n], o[:])
                off += n

    nc.compile()
    return nc


def _get_nc():
    global _cached
    if _cached is None:
        _cached = _build()
    return _cached


def _prep_in_maps(input, user_repost_matrix, W, b):
    idx_full = np.asarray(input).reshape(-1).astype(np.int64)
    table_f32 = np.asarray(user_repost_matrix, dtype=np.float32)
    W_f32 = np.asarray(W, dtype=np.float32)
    b_f32 = np.asarray(b, dtype=np.float32)

    # bf16 table, feature dim padded to 2048
    tbl = np.zeros((NTOKEN, D2), dtype=ml_dtypes.bfloat16)
    tbl[:, :D] = table_f32.astype(ml_dtypes.bfloat16)

    # W tile: w[p, c*8+j] = Wt_pad[c*128+p, j]
    wt = np.zeros((D2, J), dtype=np.float32)
    wt[:D] = W_f32.T
    w_tile = np.ascontiguousarray(
        wt.astype(ml_dtypes.bfloat16)
        .reshape(KCH, 128, J)
        .transpose(1, 0, 2)
        .reshape(128, KCH * J)
    )

    order = np.argsort(idx_full, kind="stable")
    idx_sorted = idx_full[order]

    in_maps = []
    bases = []
    oob = []                      # (core, slot) of out-of-window tokens
    for c in range(N_CORES):
        lo = c * NI
        hi = min(lo + NI, TOK)
        cnt = hi - lo
        gidx = np.empty(NI, np.int64)
        gidx[:cnt] = idx_sorted[lo:hi]
        gidx[cnt:] = gidx[cnt - 1]
        base = int(min(gidx[0], NTOKEN - S))
        loc = gidx - base
        bad = (loc < 0) | (loc >= S)
        if bad.any():
            for slot in np.nonzero(bad)[0]:
                oob.append((c, int(slot)))
            loc = np.clip(loc, 0, S - 1)
        loc16 = loc.astype(np.int16)
        # idx tile: slot i -> [g*16 + i%16, i//16], replicated over 8 groups
        idx_tile = np.tile(
            np.ascontiguousarray(loc16.reshape(NI // 16, 16).T), (8, 1)
        )
        in_maps.append(
            {
                "table": np.ascontiguousarray(tbl[base : base + S]),
                "idx": idx_tile,
                "w": w_tile,
            }
        )
        bases.append(base)

    ctx = {
        "order": order,
        "oob": oob,
        "idx_full": idx_full,
        "table_f32": table_f32,
        "W_f32": W_f32,
        "b_f32": b_f32,
    }
    return in_maps, ctx


def _run(in_maps, trace=False, **kw):
    nc = _get_nc()
    return run_bass_kernel_spmd(
        nc, in_maps, list(range(N_CORES)), trace=trace, **kw
    )


def _unshard(results, ctx):
    order = ctx["order"]
    sorted_out = np.concatenate(
        [results[c]["out"] for c in range(N_CORES)], axis=1
    )[:, :TOK].T.astype(np.float32)          # [12800, 8] in sorted order
    final = np.empty((TOK, J), np.float32)
    final[order] = sorted_out
    # host f32 fallback for tokens outside their core's staged window
    for c, slot in ctx["oob"]:
        k = c * NI + slot
        if k < TOK:
            tok = order[k]
            final[tok] = ctx["table_f32"][ctx["idx_full"][tok]] @ ctx["W_f32"].T
    final += ctx["b_f32"].reshape(1, J)
    return final.reshape(B, L, J)


def kernel(input, user_repost_matrix, W, b):
    in_maps, ctx = _prep_in_maps(input, user_repost_matrix, W, b)
    res = _run(in_maps)
    return _unshard(res.results, ctx)
